# revision 20
# baseline (speedup 1.0000x reference)
"""Multi-head attention kernel for Trainium2, sharded over 8 NeuronCores.

Sharding: core c handles batch c//4 and heads 4*(c%4) .. 4*(c%4)+4
(data parallel on batch, tensor parallel on heads).  Each core computes a
partial output projection (its heads' slice of Wout); the host sums the 4
partials per batch at gather time.

Self-contained: hardcodes B=2, S=2048, D=1024, H=16.

Schedule: causal attention concentrates exp work (ACT engine) in the last
q-chunk, which made the endgame ACT-bound while the PE idled.  The kernel
therefore runs stream (sc=3, pr=1) EARLY, interleaved one-block-per-~2.5
into chunks 0-2 (its kt/v/qt inputs arrive early enough), so the exp hump
flattens across the whole kernel.  PSUM feasibility comes from packing each
stream's two heads' PV into ONE psum bank via col-tiled M=64 matmul pairs
(tile_position (0,0)/(0,64)) with softmax denominators accumulated as M=1
col-tiled matmuls into a shared Z bank (rows 64/96 = current lane's two
heads, rows 0/32 = early lane's).  Banks: scores 2x[128,2QC]=4 + PV 2 +
Z 1 + proj 1 = 8 exactly.
"""

import numpy as np
import ml_dtypes
from contextlib import ExitStack

import concourse.bass as bass
import concourse.tile as tile
from concourse import mybir
from concourse import bass_utils as _BU
from concourse.bass_utils import run_bass_kernel_spmd

BF16 = ml_dtypes.bfloat16

B, S, D, H = 2, 2048, 1024, 16
A = 64                  # head dim
NH = 4                  # heads per core
M = NH * A              # 256: local projection width
SCALE = 1.0 / 32.0      # 1/sqrt(D)
P = 128
QC = 512                # q chunk (matmul free dim)
NQC = S // QC           # 4
KC = 128                # k chunk (contraction tile for PV)
NKC = S // KC           # 16
DC = D // P             # 8 d-chunks

F32 = mybir.dt.float32
DT16 = mybir.dt.bfloat16
EXP = mybir.ActivationFunctionType.Exp
LN = mybir.ActivationFunctionType.Ln

_prog_cache = {}


def _bcast_part(ap, n):
    """Broadcast a [1, ...] AP across n partitions (step-0 partition dim)."""
    return bass.AP(tensor=ap.tensor, offset=ap.offset, ap=[[0, n]] + list(ap.ap[1:]))


def _build(causal: bool) -> bass.Bass:
    nc = bass.Bass()

    # all inputs pre-swizzled on host into SBUF layout (contiguous per
    # partition -> large DMA descriptors -> near-peak HBM bandwidth).
    qT = nc.dram_tensor("qT", [P, NQC, DC, QC], DT16, kind="ExternalInput")
    cT = nc.dram_tensor("cT", [P, NQC, DC, QC], DT16, kind="ExternalInput")
    wqT = nc.dram_tensor("wqT", [P, DC, M], DT16, kind="ExternalInput")
    wkT = nc.dram_tensor("wkT", [P, DC, M], DT16, kind="ExternalInput")
    wvT = nc.dram_tensor("wvT", [P, DC, M], DT16, kind="ExternalInput")
    woT = nc.dram_tensor("woT", [P, 2, D], DT16, kind="ExternalInput")
    if causal:
        m01 = nc.dram_tensor("m01", [P, KC], DT16, kind="ExternalInput")
    else:
        emT = nc.dram_tensor("emT", [S, S], DT16, kind="ExternalInput")
    outT = nc.dram_tensor("outT", [D, S], DT16, kind="ExternalOutput")

    with tile.TileContext(nc) as tc, ExitStack() as ctx:
        const = ctx.enter_context(tc.tile_pool(name="const", bufs=1))

        # Persistent SBUF tensors
        qt_in = const.tile([P, DC, S], DT16, tag="qt_in")    # query^T  (d on partitions)
        ct_in = const.tile([P, DC, S], DT16, tag="ct_in")    # context^T
        wq_sb = const.tile([P, DC, M], DT16, tag="wq_sb")
        wk_sb = const.tile([P, DC, M], DT16, tag="wk_sb")
        wv_sb = const.tile([P, DC, M], DT16, tag="wv_sb")
        wo_sb = const.tile([P, 2, D], DT16, tag="wo_sb")
        qt = [const.tile([P, S], DT16, tag=f"qt{i}", name=f"qt{i}") for i in range(2)]   # Q^T m-chunks
        kt = [const.tile([P, S], DT16, tag=f"kt{i}", name=f"kt{i}") for i in range(2)]   # K^T m-chunks
        v_sb = const.tile([P, NKC, NH * (A + 1)], DT16, tag="v_sb")   # [V_h | 1] blocks
        u_sb = [const.tile([P, S], DT16, tag=f"u{i}", name=f"u{i}") for i in range(2)]  # normalized attn@V
        if causal:
            m01_sb = const.tile([P, KC], DT16, tag="m01_sb")
        ones_t = const.tile([P, A], DT16, tag="ones_t")   # lhsT for Z broadcast

        # Input DMAs, single (sync) queue in need order.  The ~6 us framework
        # preamble gates the first descriptor; after that the pieces stream at
        # full aggregate bandwidth, so granularity is 2-dc (256 KB) for the
        # sc0 slabs that gate the first projections.  q3 is pulled forward:
        # the early lane (3,1) starts a few blocks in.
        def q_slab(eng, sc_, lo, hi):
            return eng.dma_start(out=qt_in[:, lo:hi, sc_ * QC:(sc_ + 1) * QC],
                                 in_=qT[:, sc_, lo:hi, :])

        def c_slab(eng, sc_, lo, hi):
            return eng.dma_start(out=ct_in[:, lo:hi, sc_ * QC:(sc_ + 1) * QC],
                                 in_=cT[:, sc_, lo:hi, :])

        if causal:
            nc.sync.dma_start(out=m01_sb[:], in_=m01[:, :])
        nc.sync.dma_start(out=wq_sb[:, 0:4, :], in_=wqT[:, 0:4, :])
        q_slab(nc.sync, 0, 0, 2)
        q_slab(nc.sync, 0, 2, 4)
        nc.sync.dma_start(out=wq_sb[:, 4:8, :], in_=wqT[:, 4:8, :])
        nc.sync.dma_start(out=wk_sb[:], in_=wkT[:, :, :])
        q_slab(nc.sync, 0, 4, 6)
        c_slab(nc.sync, 0, 0, 2)
        q_slab(nc.sync, 0, 6, 8)
        c_slab(nc.sync, 0, 2, 4)
        c_slab(nc.sync, 0, 4, 6)
        c_slab(nc.sync, 0, 6, 8)
        nc.sync.dma_start(out=wv_sb[:], in_=wvT[:, :, :])
        q_slab(nc.sync, 3, 0, 8)
        c_slab(nc.sync, 1, 0, 4)
        c_slab(nc.sync, 1, 4, 8)
        q_slab(nc.sync, 1, 0, 8)
        nc.sync.dma_start(out=wo_sb[:], in_=woT[:, :, :])
        c_slab(nc.sync, 2, 0, 8)
        q_slab(nc.sync, 2, 0, 8)
        c_slab(nc.sync, 3, 0, 8)

        nc.vector.memset(v_sb[:], 1.0)  # ones columns for the Z (denominator) trick
        nc.vector.memset(ones_t[:], 1.0)

        # PSUM: scores 2x[128,2QC] = 4 banks + per-lane PV pairs 4 banks = 8.
        # Projections and out-projections borrow scores-pool banks (the ring
        # serializes them ~2 allocations apart, past the exp-read latency).
        with tc.tile_pool(name="ps_s", bufs=2, space="PSUM") as ps_s_pool, \
             tc.tile_pool(name="ps_pv", bufs=1, space="PSUM") as ps_pv_pool, \
             tc.tile_pool(name="expool", bufs=12) as ex_pool, \
             tc.tile_pool(name="zdram", bufs=4, space="DRAM") as zd_pool, \
             tc.tile_pool(name="o_stage", bufs=8) as o_stage, \
             tc.tile_pool(name="norm", bufs=4) as norm_pool:

            def _proj_ps(alt=False):
                return ps_s_pool.tile([P, 2 * QC], F32, tag="ps_s",
                                      name="ps_alt")[:, 0:QC]

            def q_proj(mc, sc, alt=False):
                ps = _proj_ps(alt)
                for dc_ in range(DC):
                    nc.tensor.matmul(
                        ps[:, 0:QC],
                        lhsT=wq_sb[:, dc_, mc * P:(mc + 1) * P],
                        rhs=qt_in[:, dc_, sc * QC:(sc + 1) * QC],
                        start=(dc_ == 0), stop=(dc_ == DC - 1),
                    )
                nc.vector.tensor_copy(out=qt[mc][:, sc * QC:(sc + 1) * QC], in_=ps[:, 0:QC])

            def k_proj(mc, sc, alt=False):
                ps = _proj_ps(alt)
                for dc_ in range(DC):
                    nc.tensor.matmul(
                        ps[:, 0:QC],
                        lhsT=wk_sb[:, dc_, mc * P:(mc + 1) * P],
                        rhs=ct_in[:, dc_, sc * QC:(sc + 1) * QC],
                        start=(dc_ == 0), stop=(dc_ == DC - 1),
                    )
                nc.vector.tensor_copy(out=kt[mc][:, sc * QC:(sc + 1) * QC], in_=ps[:, 0:QC])

            def v_proj(cc, alt=False):
                ps = _proj_ps(alt)
                for dc_ in range(DC):
                    nc.tensor.matmul(
                        ps[:, 0:M],
                        lhsT=ct_in[:, dc_, cc * P:(cc + 1) * P],
                        rhs=wv_sb[:, dc_, :],
                        start=(dc_ == 0), stop=(dc_ == DC - 1),
                    )
                for h in range(NH):
                    nc.vector.tensor_copy(
                        out=v_sb[:, cc, h * (A + 1):h * (A + 1) + A],
                        in_=ps[:, h * A:(h + 1) * A],
                    )

            def o_evac(jc, sc, ps, off=0, act_ok=False):
                ps = ps[:, off:off + QC]
                o_sb = o_stage.tile([P, QC], DT16, tag="o_sb")
                if act_ok:
                    # late windows: ACT has slack, DVE is the busier engine
                    if jc % 2 == 0:
                        nc.scalar.copy(out=o_sb[:], in_=ps[:, 0:QC])
                    else:
                        nc.vector.tensor_copy(out=o_sb[:], in_=ps[:, 0:QC])
                    deng = nc.sync if jc % 2 == 0 else nc.scalar
                else:
                    nc.vector.tensor_copy(out=o_sb[:], in_=ps[:, 0:QC])
                    deng = nc.sync if jc % 2 == 0 else nc.gpsimd
                deng.dma_start(
                    out=outT[:, :][jc * P:(jc + 1) * P, sc * QC:(sc + 1) * QC],
                    in_=o_sb[:])

            def out_proj(jc, sc):
                ps = _proj_ps()
                for ic in range(2):
                    nc.tensor.matmul(
                        ps[:, 0:QC],
                        lhsT=wo_sb[:, ic, jc * P:(jc + 1) * P],
                        rhs=u_sb[ic][:, sc * QC:(sc + 1) * QC],
                        start=(ic == 0), stop=(ic == 1),
                    )
                o_evac(jc, sc, ps, act_ok=(sc >= 2))

            # ---- attention blocks -------------------------------------------
            # scores: row-tiled pair (contraction A=64; lhsT base partitions 0
            # and 64 auto-derive tile_position (0,0)/(64,0)) -> two psum banks
            # of one [P, 2QC] scores tile, running concurrently in the array.
            # PV: col-tiled M=64 pair into ONE bank (rows 0:64 head h0, 64:128
            # h1).  Z: col-tiled M=1 pair into the shared Z bank.
            def attn_scores(pr, sc, kc_, nkc):
                r = kc_ - 4 * sc
                w0 = KC * r if (causal and r > 0) else 0
                ps = ps_s_pool.tile([P, 2 * QC], F32, tag="ps_s", name="ps_s")
                nc.tensor.matmul(
                    ps[:, w0:QC],
                    lhsT=kt[pr][0:A, kc_ * KC:(kc_ + 1) * KC],
                    rhs=qt[pr][0:A, sc * QC + w0:(sc + 1) * QC],
                    start=True, stop=True,
                )
                nc.tensor.matmul(
                    ps[:, QC + w0:2 * QC],
                    lhsT=kt[pr][A:2 * A, kc_ * KC:(kc_ + 1) * KC],
                    rhs=qt[pr][A:2 * A, sc * QC + w0:(sc + 1) * QC],
                    start=True, stop=True,
                )
                ex = ex_pool.tile([P, 2 * QC], DT16, tag="ex", name="ex")
                if w0 == 0:
                    nc.scalar.activation(out=ex[:], in_=ps[:],
                                         func=EXP, scale=SCALE)
                else:
                    nc.scalar.activation(out=ex[:, w0:QC], in_=ps[:, w0:QC],
                                         func=EXP, scale=SCALE)
                    nc.scalar.activation(out=ex[:, QC + w0:2 * QC],
                                         in_=ps[:, QC + w0:2 * QC],
                                         func=EXP, scale=SCALE)
                if causal:
                    if r >= 0:  # mask the 128-wide boundary band only
                        nc.vector.tensor_mul(
                            ex[:, w0:w0 + KC], ex[:, w0:w0 + KC], m01_sb[:])
                        nc.vector.tensor_mul(
                            ex[:, QC + w0:QC + w0 + KC],
                            ex[:, QC + w0:QC + w0 + KC], m01_sb[:])
                else:
                    em = ex_pool.tile([P, QC], DT16, tag="em", name="em")
                    nc.sync.dma_start(
                        out=em[:],
                        in_=emT[:, :][kc_ * KC:(kc_ + 1) * KC,
                                      sc * QC:(sc + 1) * QC],
                    )
                    nc.vector.tensor_mul(ex[:, 0:QC], ex[:, 0:QC], em[:])
                    nc.vector.tensor_mul(ex[:, QC:2 * QC], ex[:, QC:2 * QC], em[:])
                return ex, w0

            def attn_pvz(pr, sc, kc_, nkc, pvA, pvB, ex, w0):
                # PV with ones-column (psum row A holds Z).  Non-tiled M=65
                # matmuls: their LDWEIGHTS hoist through the PE reorder
                # window (tiled matmuls' can't -- weight-slot conflicts cost
                # ~100ns/pass exposed), so serial M=65 beats a col-tiled
                # M=64 pair plus a separate Z pass.
                st, sp = (kc_ == 0), (kc_ == nkc - 1)
                h0, h1 = 2 * pr, 2 * pr + 1
                nc.tensor.matmul(
                    pvA[0:A + 1, w0:QC],
                    lhsT=v_sb[:, kc_, h0 * (A + 1):(h0 + 1) * (A + 1)],
                    rhs=ex[:, w0:QC],
                    start=st, stop=sp,
                )
                nc.tensor.matmul(
                    pvB[0:A + 1, w0:QC],
                    lhsT=v_sb[:, kc_, h1 * (A + 1):(h1 + 1) * (A + 1)],
                    rhs=ex[:, QC + w0:2 * QC],
                    start=st, stop=sp,
                )

            tail_ps = []    # ic1-half psums started during the tail normalize

            def tail_ic1(pvs_free):
                # Start the u_sb[1]-half of ALL eight tail out-projections
                # while the last normalize's Z chain runs (u_sb[1] is the
                # early lane, normalized long ago).  Banks: two scores tiles
                # (2 banks each, via independent column halves) + the early
                # lane's PV pair + the tail stream's own pvA; its pvB serves
                # as the normalize's broadcast bank first, then joins as the
                # eighth after the Ln pass frees it (handled by the caller).
                sc = NQC - 1
                sh = {}
                for jc in range(7):
                    off = 0
                    if jc in (0, 1):
                        ps = ps_s_pool.tile([P, 2 * QC], F32, tag="ps_s",
                                            name="ps_e")
                        sh[jc + 4] = ps
                    elif jc in (4, 5):
                        ps = sh[jc]
                        off = QC
                    else:  # 2, 3, 6: early pvA/pvB, tail pvA
                        ps = pvs_free[{2: 0, 3: 1, 6: 2}[jc]]
                    nc.tensor.matmul(
                        ps[:, off:off + QC],
                        lhsT=wo_sb[:, 1, jc * P:(jc + 1) * P],
                        rhs=u_sb[1][:, sc * QC:(sc + 1) * QC],
                        start=True, stop=False,
                    )
                    tail_ps.append((ps, off))

            def normalize(pr, sc, pvA, pvB, tail=False, pv_early=None):
                if tail:
                    # Critical-path variant: no DRAM bounce.  Evict u halves
                    # on ACT, broadcast Z down 64 partitions with a ones-lhsT
                    # matmul (into the freed pvB bank), 1/Z = exp(-ln Z) on
                    # ACT, multiply.
                    nc.scalar.copy(
                        out=u_sb[pr][0:A, sc * QC:(sc + 1) * QC], in_=pvA[0:A, :])
                    bt = norm_pool.tile([A, QC], DT16, tag="bt", name="bt")
                    nc.scalar.copy(out=bt[:], in_=pvB[0:A, :])
                    nc.scalar.dma_start(
                        out=u_sb[pr][A:2 * A, sc * QC:(sc + 1) * QC], in_=bt[:])
                    zr = norm_pool.tile([P, 2 * QC], DT16, tag="zr", name="zr")
                    nc.vector.tensor_copy(out=zr[A:A + 1, 0:QC], in_=pvA[A:A + 1, :])
                    nc.vector.tensor_copy(out=zr[A:A + 1, QC:2 * QC],
                                          in_=pvB[A:A + 1, :])
                    rbp = pvB[:, 0:QC]
                    nc.tensor.matmul(rbp[0:A, :], lhsT=ones_t[A:A + 1, 0:A],
                                     rhs=zr[A:A + 1, 0:QC], start=True, stop=True)
                    nc.tensor.matmul(rbp[A:2 * A, :], lhsT=ones_t[A:A + 1, 0:A],
                                     rhs=zr[A:A + 1, QC:2 * QC], start=True, stop=True)
                    lnz = norm_pool.tile([P, QC], F32, tag="lnz", name="lnz")
                    nc.scalar.activation(out=lnz[:], in_=rbp[:], func=LN)
                    rb = norm_pool.tile([P, QC], F32, tag="rb", name="rb")
                    nc.scalar.activation(out=rb[:], in_=lnz[:],
                                         func=EXP, scale=-1.0)
                    tail_ic1((pv_early[0], pv_early[1], pvA))
                    # jc7 reuses the broadcast bank once Ln has read it
                    nc.tensor.matmul(
                        pvB[:, 0:QC],
                        lhsT=wo_sb[:, 1, 7 * P:8 * P],
                        rhs=u_sb[1][:, sc * QC:(sc + 1) * QC],
                        start=True, stop=False,
                    )
                    tail_ps.append((pvB, 0))
                    nc.vector.tensor_mul(
                        u_sb[pr][:, sc * QC:(sc + 1) * QC],
                        u_sb[pr][:, sc * QC:(sc + 1) * QC], rb[:])
                    return
                # Evict U unnormalized (frees PV psum fast); 1/Z via DRAM
                # bounce reshaped [128, 8] (lane-parallel reciprocal), then
                # one in-place multiply.  ACT stays exp-only.
                zr = norm_pool.tile([P, 2 * QC], F32, tag="zf", name="zf")
                nc.vector.tensor_copy(out=zr[A:A + 1, 0:QC], in_=pvA[A:A + 1, :])
                nc.vector.tensor_copy(out=zr[A:A + 1, QC:2 * QC], in_=pvB[A:A + 1, :])
                nc.vector.tensor_copy(
                    out=u_sb[pr][0:A, sc * QC:(sc + 1) * QC], in_=pvA[0:A, :])
                bt = norm_pool.tile([A, QC], DT16, tag="bt", name="bt")
                nc.vector.tensor_copy(out=bt[:], in_=pvB[0:A, :])
                nc.gpsimd.dma_start(
                    out=u_sb[pr][A:2 * A, sc * QC:(sc + 1) * QC], in_=bt[:])
                zd = zd_pool.tile([1, 2 * QC], F32, tag="zd", name="zd")
                nc.sync.dma_start(out=zd[:], in_=zr[A:A + 1, :])
                zre = bass.AP(tensor=zd.tensor, offset=zd.offset,
                              ap=[[8, P], [1, 8]])
                zi = norm_pool.tile([P, 8], F32, tag="zi", name="zi")
                nc.sync.dma_start(out=zi[:], in_=zre)
                nc.vector.reciprocal(out=zi[:], in_=zi[:])
                zd2 = zd_pool.tile([1, 2 * QC], F32, tag="zd2", name="zd2")
                zre2 = bass.AP(tensor=zd2.tensor, offset=zd2.offset,
                               ap=[[8, P], [1, 8]])
                nc.sync.dma_start(out=zre2, in_=zi[:])
                rb = norm_pool.tile([P, QC], F32, tag="rb", name="rb")
                nc.sync.dma_start(out=rb[0:A, :], in_=_bcast_part(zd2[0:1, 0:QC], A))
                nc.sync.dma_start(out=rb[A:2 * A, :],
                                  in_=_bcast_part(zd2[0:1, QC:2 * QC], A))
                nc.vector.tensor_mul(
                    u_sb[pr][:, sc * QC:(sc + 1) * QC],
                    u_sb[pr][:, sc * QC:(sc + 1) * QC], rb[:])

            # ---- schedule ---------------------------------------------------
            # cur lane: pr-major ascending chunks, ending with (3,0).
            # early lane: (3,1), one block per ~2.5 cur positions.
            if causal:
                cur_streams = [(0, 0), (0, 1), (1, 0), (1, 1), (2, 0), (2, 1),
                               (3, 0)]
                early_stream = (3, 1)
                nkc_of = lambda sc: 4 * sc + 4
            else:
                cur_streams = [(0, 0), (0, 1), (1, 0), (1, 1), (2, 0), (2, 1),
                               (3, 0)]
                early_stream = (3, 1)
                nkc_of = lambda sc: NKC

            cur_blocks = []
            for sc, pr in cur_streams:
                for kc_ in range(nkc_of(sc)):
                    cur_blocks.append((sc, pr, kc_))
            n_pos = len(cur_blocks)   # 64 causal / 112 non-causal

            e_nkc = nkc_of(early_stream[0])
            if causal:
                # hand-placed to avoid the positions carrying heavy fill
                early_at = {p: i for i, p in enumerate(
                    (6, 8, 11, 13, 17, 19, 22, 25, 27, 30,
                     33, 35, 37, 40, 42, 44))}
            else:
                # spread evenly over the first ~3/4 of the schedule
                early_at = {8 + (i * 3 * n_pos) // (4 * e_nkc): i
                            for i in range(e_nkc)}

            # fill units keyed by target cur position (emitted right after
            # that position's block).  Deadlines honored; oproj(sc) deferred
            # one-or-more chunks to plug the later ACT-heavy windows.
            fill_at = {}

            def put(pos, fn):
                fill_at.setdefault(pos, []).append(fn)

            if causal:
                put(0, lambda: v_proj(2))
                put(1, lambda: v_proj(3))
                put(2, lambda: q_proj(1, 0))
                put(3, lambda: k_proj(1, 0))
                put(4, lambda: q_proj(1, 3))       # early lane q
                put(5, lambda: q_proj(0, 1))
                put(7, lambda: k_proj(0, 1))
                put(9, lambda: v_proj(4))
                put(10, lambda: v_proj(5))
                put(12, lambda: v_proj(6))
                put(14, lambda: v_proj(7))
                put(15, lambda: q_proj(1, 1))
                put(15, lambda: k_proj(1, 1))
                put(18, lambda: q_proj(0, 2))
                put(20, lambda: k_proj(0, 2))
                put(21, lambda: out_proj(0, 0))
                put(23, lambda: out_proj(1, 0))
                put(24, lambda: k_proj(1, 2))
                put(26, lambda: v_proj(8))
                put(28, lambda: v_proj(9))
                put(29, lambda: v_proj(10))
                put(31, lambda: v_proj(11))
                put(32, lambda: q_proj(1, 2))
                put(34, lambda: k_proj(1, 3))      # early lane k, chunk 3
                put(36, lambda: v_proj(12))
                put(38, lambda: v_proj(13))
                put(39, lambda: v_proj(14))
                put(41, lambda: v_proj(15))
                put(43, lambda: out_proj(2, 0))
                put(45, lambda: q_proj(0, 3))
                put(45, lambda: out_proj(3, 0))
                put(46, lambda: out_proj(4, 0))
                put(47, lambda: out_proj(5, 0))
                put(48, lambda: out_proj(6, 0))
                put(49, lambda: out_proj(7, 0))
                for i in range(8):
                    put(50 + i, lambda jc=i: out_proj(jc, 1))
                put(56, lambda: k_proj(0, 3))
                for i in range(7):
                    put(57 + i, lambda jc=i: out_proj(jc, 2))
                put(63, lambda: out_proj(7, 2))
            else:
                # non-causal correctness path: everything is projected in the
                # prologue (below); only the out-projections ride as fill.
                for jc in range(8):
                    put(32 + jc, lambda jc=jc: out_proj(jc, 0))
                for jc in range(8):
                    put(48 + jc, lambda jc=jc: out_proj(jc, 1))
                for jc in range(8):
                    put(80 + jc, lambda jc=jc: out_proj(jc, 2))

            # Prologue: only what the first blocks need.
            q_proj(0, 0)
            k_proj(0, 0, alt=True)
            v_proj(0)
            v_proj(1, alt=True)
            if not causal:
                for nsc in range(NQC):
                    for mc in range(2):
                        if (mc, nsc) != (0, 0):
                            q_proj(mc, nsc, alt=(nsc % 2 == 0))
                            k_proj(mc, nsc, alt=(nsc % 2 == 1))
                for cc in range(2, NKC):
                    v_proj(cc, alt=(cc % 2 == 0))

            pv_of = {}       # stream -> (pvA, pvB) psum tiles
            pending = []     # deferred PV/normalize closures (FIFO)

            def emit_block(stream, kc_, lane):
                sc, pr = stream
                nkc = nkc_of(sc)
                if kc_ == 0:
                    pv_of[stream] = (
                        ps_pv_pool.tile([P, QC], F32, tag=f"pv{lane}A",
                                        name=f"pv{lane}A"),
                        ps_pv_pool.tile([P, QC], F32, tag=f"pv{lane}B",
                                        name=f"pv{lane}B"),
                    )
                pvA, pvB = pv_of[stream]
                ex, w0 = attn_scores(pr, sc, kc_, nkc)
                pending.append(
                    lambda: attn_pvz(pr, sc, kc_, nkc, pvA, pvB, ex, w0))
                if kc_ == nkc - 1:
                    is_tail = (lane == "C" and stream == cur_streams[-1])
                    if is_tail:
                        pending.append(
                            lambda: normalize(pr, sc, pvA, pvB, tail=True,
                                              pv_early=pv_of[early_stream]))
                    else:
                        pending.append(
                            lambda: normalize(pr, sc, pvA, pvB))

            for pos, (sc, pr, kc_) in enumerate(cur_blocks):
                emit_block((sc, pr), kc_, "C")
                if pos in early_at:
                    emit_block(early_stream, early_at[pos], "E")
                while len(pending) > 3:
                    pending.pop(0)()
                for fn in fill_at.get(pos, ()):
                    fn()
            while pending:
                pending.pop(0)()

            # Tail: finish the eight out-projections with the u_sb[0] half.
            for jc in range(8):
                ps, off = tail_ps[jc]
                nc.tensor.matmul(
                    ps[:, off:off + QC],
                    lhsT=wo_sb[:, 0, jc * P:(jc + 1) * P],
                    rhs=u_sb[0][:, (NQC - 1) * QC:NQC * QC],
                    start=False, stop=True,
                )
                o_evac(jc, NQC - 1, ps, off, act_ok=True)

    return nc


def _split_waits(nc: bass.Bass) -> int:
    """The walrus build here allows one sync wait per engine instruction;
    Tile emits several.  Hoist extras into standalone single-wait
    EventSemaphore instructions on the same engine queue (in-order, so
    semantics are preserved).  DMACopy waits lower into queue descriptors and
    are left alone."""
    n = 0
    for func in nc.m.functions:
        for block in func.blocks:
            out = []
            for ins in block.instructions:
                si = ins.sync_info
                if si is not None and len(si.on_wait) > 1:
                    waits = list(si.on_wait)
                    for w in waits[:-1]:
                        es = mybir.InstEventSemaphore(
                            name=f"waitsplit_{n}", ins=[], outs=[])
                        n += 1
                        es.engine = ins.engine
                        es.sync_info = type(si)(on_wait=[w], on_update=[])
                        out.append(es)
                    si.on_wait = [waits[-1]]
                    ins.sync_info = si
                out.append(ins)
            block.instructions = out
    return n


def _get_prog(causal: bool) -> bass.Bass:
    if causal not in _prog_cache:
        nc = _build(causal)
        _split_waits(nc)
        _prog_cache[causal] = nc
    return _prog_cache[causal]


def _is_causal(mask: np.ndarray) -> bool:
    if mask.shape != (S, S):
        return False
    tri = np.tril(np.ones((S, S), dtype=bool))
    low = mask[tri]
    up = mask[~tri]
    return bool((low == 0.0).all() and (up <= -1e8).all())


def _m01_patterns() -> np.ndarray:
    # Boundary-band mask: band column j vs partition p -> keep iff j >= p.
    j = np.arange(KC)[None, :]
    p = np.arange(P)[:, None]
    return (j >= p).astype(BF16)


def _prep_in_maps(query, context, Wq, Wkv, Wout, mask, causal):
    query = np.asarray(query, dtype=np.float32)
    context = np.asarray(context, dtype=np.float32)
    Wq = np.asarray(Wq, dtype=np.float32)
    Wkv = np.asarray(Wkv, dtype=np.float32)
    Wout = np.asarray(Wout, dtype=np.float32)

    def sw_act(x):   # [D, S] -> [P, NQC, DC, QC] (sc-major SBUF-layout swizzle)
        return np.ascontiguousarray(
            x.reshape(DC, P, NQC, QC).transpose(1, 2, 0, 3)).astype(BF16)

    def sw_w(w):     # [D, M] -> [P, DC, M]
        return np.ascontiguousarray(
            w.reshape(DC, P, M).transpose(1, 0, 2)).astype(BF16)

    def sw_wo(w):    # [M, D] -> [P, 2, D]
        return np.ascontiguousarray(
            w.reshape(2, P, D).transpose(1, 0, 2)).astype(BF16)

    qT = [sw_act(query[b].T) for b in range(B)]
    cT = [sw_act(context[b].T) for b in range(B)]
    if causal:
        extra = ("m01", _m01_patterns())
    else:
        extra = ("emT", np.exp((SCALE * np.asarray(mask, np.float32).T)).astype(BF16))

    in_maps = []
    for c in range(8):
        b, g = divmod(c, 4)
        m0 = g * M
        in_maps.append({
            "qT": qT[b],
            "cT": cT[b],
            "wqT": sw_w(Wq[m0:m0 + M, :].T),
            "wkT": sw_w(Wkv[m0:m0 + M, :].T),
            "wvT": sw_w(Wkv[D + m0:D + m0 + M, :].T),
            "woT": sw_wo(Wout[:, m0:m0 + M].T),
            extra[0]: extra[1],
        })
    return in_maps


def _run(query, context, Wq, Wkv, Wout, mask, trace=False):
    causal = _is_causal(np.asarray(mask, np.float32))
    in_maps = _prep_in_maps(query, context, Wq, Wkv, Wout, mask, causal)
    nc = _get_prog(causal)
    res = run_bass_kernel_spmd(nc, in_maps, list(range(8)), trace=trace)
    out = np.zeros((B, S, D), dtype=np.float32)
    for c in range(8):
        out[c // 4] += res.results[c]["outT"].T.astype(np.float32)
    return out, res


def kernel(query, context, Wq, Wkv, Wout, mask):
    out, _ = _run(query, context, Wq, Wkv, Wout, mask, trace=False)
    return out


# revision 28
# speedup vs baseline: 1.1803x; 1.1803x over previous
"""Multi-head attention kernel for Trainium2, sharded over 8 NeuronCores.

Sharding: core c handles batch c//4 and heads 4*(c%4) .. 4*(c%4)+4
(data parallel on batch, tensor parallel on heads).  Each core computes a
partial output projection (its heads' slice of Wout); the host sums the 4
partials per batch at gather time.

Self-contained: hardcodes B=2, S=2048, D=1024, H=16.

Schedule: causal attention concentrates exp work (ACT engine) in the last
q-chunk, which made the endgame ACT-bound while the PE idled.  The kernel
therefore runs stream (sc=3, pr=1) EARLY, interleaved one-block-per-~2.5
into chunks 0-2 (its kt/v/qt inputs arrive early enough), so the exp hump
flattens across the whole kernel.  PSUM feasibility comes from packing each
stream's two heads' PV into ONE psum bank via col-tiled M=64 matmul pairs
(tile_position (0,0)/(0,64)) with softmax denominators accumulated as M=1
col-tiled matmuls into a shared Z bank (rows 64/96 = current lane's two
heads, rows 0/32 = early lane's).  Banks: scores 2x[128,2QC]=4 + PV 2 +
Z 1 + proj 1 = 8 exactly.
"""

import numpy as np
import ml_dtypes
from contextlib import ExitStack

import concourse.bass as bass
import concourse.tile as tile
from concourse import mybir
from concourse import bass_utils as _BU
from concourse.bass_utils import run_bass_kernel_spmd

BF16 = ml_dtypes.bfloat16

B, S, D, H = 2, 2048, 1024, 16
A = 64                  # head dim
NH = 4                  # heads per core
M = NH * A              # 256: local projection width
SCALE = 1.0 / 32.0      # 1/sqrt(D)
P = 128
QC = 512                # q chunk (matmul free dim)
NQC = S // QC           # 4
KC = 128                # k chunk (contraction tile for PV)
NKC = S // KC           # 16
DC = D // P             # 8 d-chunks

F32 = mybir.dt.float32
DT16 = mybir.dt.bfloat16
EXP = mybir.ActivationFunctionType.Exp
LN = mybir.ActivationFunctionType.Ln

_prog_cache = {}


def _bcast_part(ap, n):
    """Broadcast a [1, ...] AP across n partitions (step-0 partition dim)."""
    return bass.AP(tensor=ap.tensor, offset=ap.offset, ap=[[0, n]] + list(ap.ap[1:]))


def _build(causal: bool) -> bass.Bass:
    nc = bass.Bass()

    # all inputs pre-swizzled on host into SBUF layout (contiguous per
    # partition -> large DMA descriptors -> near-peak HBM bandwidth).
    qT = nc.dram_tensor("qT", [P, NQC, DC, QC], DT16, kind="ExternalInput")
    cT = nc.dram_tensor("cT", [P, NQC, DC, QC], DT16, kind="ExternalInput")
    wqT = nc.dram_tensor("wqT", [P, DC, M], DT16, kind="ExternalInput")
    wkT = nc.dram_tensor("wkT", [P, DC, M], DT16, kind="ExternalInput")
    wvT = nc.dram_tensor("wvT", [P, DC, M], DT16, kind="ExternalInput")
    woT = nc.dram_tensor("woT", [P, 2, D], DT16, kind="ExternalInput")
    if causal:
        m01 = nc.dram_tensor("m01", [P, KC], DT16, kind="ExternalInput")
    else:
        emT = nc.dram_tensor("emT", [S, S], DT16, kind="ExternalInput")
    outT = nc.dram_tensor("outT", [D, S], DT16, kind="ExternalOutput")

    with tile.TileContext(nc) as tc, ExitStack() as ctx:
        const = ctx.enter_context(tc.tile_pool(name="const", bufs=1))

        # Persistent SBUF tensors
        qt_in = const.tile([P, DC, S], DT16, tag="qt_in")    # query^T  (d on partitions)
        ct_in = const.tile([P, DC, S], DT16, tag="ct_in")    # context^T
        wq_sb = const.tile([P, DC, M], DT16, tag="wq_sb")
        wk_sb = const.tile([P, DC, M], DT16, tag="wk_sb")
        wv_sb = const.tile([P, DC, M], DT16, tag="wv_sb")
        wo_sb = const.tile([P, 2, D], DT16, tag="wo_sb")
        qt = [const.tile([P, S], DT16, tag=f"qt{i}", name=f"qt{i}") for i in range(2)]   # Q^T m-chunks
        kt = [const.tile([P, S], DT16, tag=f"kt{i}", name=f"kt{i}") for i in range(2)]   # K^T m-chunks
        v_sb = const.tile([P, NKC, NH * (A + 1)], DT16, tag="v_sb")   # [V_h | 1] blocks
        u_sb = [const.tile([P, S], DT16, tag=f"u{i}", name=f"u{i}") for i in range(2)]  # normalized attn@V
        if causal:
            m01_sb = const.tile([P, KC], DT16, tag="m01_sb")
        ones_t = const.tile([P, A], DT16, tag="ones_t")   # lhsT for Z broadcast

        # Input DMAs, single (sync) queue in need order.  The ~6 us framework
        # preamble gates the first descriptor; after that the pieces stream at
        # full aggregate bandwidth, so granularity is 2-dc (256 KB) for the
        # sc0 slabs that gate the first projections.  q3 is pulled forward:
        # the early lane (3,1) starts a few blocks in.
        def q_slab(eng, sc_, lo, hi):
            return eng.dma_start(out=qt_in[:, lo:hi, sc_ * QC:(sc_ + 1) * QC],
                                 in_=qT[:, sc_, lo:hi, :])

        def c_slab(eng, sc_, lo, hi):
            return eng.dma_start(out=ct_in[:, lo:hi, sc_ * QC:(sc_ + 1) * QC],
                                 in_=cT[:, sc_, lo:hi, :])

        if causal:
            nc.sync.dma_start(out=m01_sb[:], in_=m01[:, :])
        nc.sync.dma_start(out=wq_sb[:, 0:4, :], in_=wqT[:, 0:4, :])
        q_slab(nc.sync, 0, 0, 2)
        q_slab(nc.sync, 0, 2, 4)
        nc.sync.dma_start(out=wq_sb[:, 4:8, :], in_=wqT[:, 4:8, :])
        nc.sync.dma_start(out=wk_sb[:], in_=wkT[:, :, :])
        q_slab(nc.sync, 0, 4, 6)
        c_slab(nc.sync, 0, 0, 2)
        q_slab(nc.sync, 0, 6, 8)
        c_slab(nc.sync, 0, 2, 4)
        c_slab(nc.sync, 0, 4, 6)
        c_slab(nc.sync, 0, 6, 8)
        nc.sync.dma_start(out=wv_sb[:], in_=wvT[:, :, :])
        q_slab(nc.sync, 3, 0, 8)
        c_slab(nc.sync, 1, 0, 4)
        c_slab(nc.sync, 1, 4, 8)
        q_slab(nc.sync, 1, 0, 8)
        nc.sync.dma_start(out=wo_sb[:], in_=woT[:, :, :])
        c_slab(nc.sync, 2, 0, 8)
        q_slab(nc.sync, 2, 0, 8)
        c_slab(nc.sync, 3, 0, 8)

        nc.vector.memset(v_sb[:], 1.0)  # ones columns for the Z (denominator) trick
        nc.vector.memset(ones_t[:], 1.0)

        # PSUM: scores 2x[128,2QC] = 4 banks + PV pair 2 + proj 2 = 8.
        with tc.tile_pool(name="ps_proj", bufs=2, space="PSUM") as ps_proj, \
             tc.tile_pool(name="ps_s", bufs=2, space="PSUM") as ps_s_pool, \
             tc.tile_pool(name="ps_pv", bufs=1, space="PSUM") as ps_pv_pool, \
             tc.tile_pool(name="expool", bufs=12) as ex_pool, \
             tc.tile_pool(name="zdram", bufs=4, space="DRAM") as zd_pool, \
             tc.tile_pool(name="o_stage", bufs=8) as o_stage, \
             tc.tile_pool(name="norm", bufs=4) as norm_pool:

            def _proj_ps(alt=False):
                # alt: prologue units borrow an idle scores-pool bank so the
                # first q/k/v projections overlap instead of serializing on
                # the two proj banks
                if alt:
                    return ps_s_pool.tile([P, 2 * QC], F32, tag="ps_s",
                                          name="ps_alt")[:, 0:QC]
                return ps_proj.tile([P, QC], F32, tag="ps_p", name="ps_p")

            def q_proj(mc, sc, alt=False):
                ps = _proj_ps(alt)
                for dc_ in range(DC):
                    nc.tensor.matmul(
                        ps[:, 0:QC],
                        lhsT=wq_sb[:, dc_, mc * P:(mc + 1) * P],
                        rhs=qt_in[:, dc_, sc * QC:(sc + 1) * QC],
                        start=(dc_ == 0), stop=(dc_ == DC - 1),
                    )
                nc.vector.tensor_copy(out=qt[mc][:, sc * QC:(sc + 1) * QC], in_=ps[:, 0:QC])

            def k_proj(mc, sc, alt=False):
                ps = _proj_ps(alt)
                for dc_ in range(DC):
                    nc.tensor.matmul(
                        ps[:, 0:QC],
                        lhsT=wk_sb[:, dc_, mc * P:(mc + 1) * P],
                        rhs=ct_in[:, dc_, sc * QC:(sc + 1) * QC],
                        start=(dc_ == 0), stop=(dc_ == DC - 1),
                    )
                nc.vector.tensor_copy(out=kt[mc][:, sc * QC:(sc + 1) * QC], in_=ps[:, 0:QC])

            def v_proj(cc, alt=False):
                ps = _proj_ps(alt)
                for dc_ in range(DC):
                    nc.tensor.matmul(
                        ps[:, 0:M],
                        lhsT=ct_in[:, dc_, cc * P:(cc + 1) * P],
                        rhs=wv_sb[:, dc_, :],
                        start=(dc_ == 0), stop=(dc_ == DC - 1),
                    )
                for h in range(NH):
                    nc.vector.tensor_copy(
                        out=v_sb[:, cc, h * (A + 1):h * (A + 1) + A],
                        in_=ps[:, h * A:(h + 1) * A],
                    )

            def o_evac(jc, sc, ps, off=0, act_ok=False):
                ps = ps[:, off:off + QC]
                o_sb = o_stage.tile([P, QC], DT16, tag="o_sb")
                if act_ok:
                    # late windows: ACT has slack, DVE is the busier engine
                    if jc % 2 == 0:
                        nc.scalar.copy(out=o_sb[:], in_=ps[:, 0:QC])
                    else:
                        nc.vector.tensor_copy(out=o_sb[:], in_=ps[:, 0:QC])
                    deng = nc.sync if jc % 2 == 0 else nc.scalar
                else:
                    nc.vector.tensor_copy(out=o_sb[:], in_=ps[:, 0:QC])
                    deng = nc.sync if jc % 2 == 0 else nc.gpsimd
                deng.dma_start(
                    out=outT[:, :][jc * P:(jc + 1) * P, sc * QC:(sc + 1) * QC],
                    in_=o_sb[:])

            def out_proj(jc, sc):
                ps = _proj_ps()
                for ic in range(2):
                    nc.tensor.matmul(
                        ps[:, 0:QC],
                        lhsT=wo_sb[:, ic, jc * P:(jc + 1) * P],
                        rhs=u_sb[ic][:, sc * QC:(sc + 1) * QC],
                        start=(ic == 0), stop=(ic == 1),
                    )
                o_evac(jc, sc, ps, act_ok=(sc >= 2))

            # ---- attention blocks -------------------------------------------
            # scores: row-tiled pair (contraction A=64; lhsT base partitions 0
            # and 64 auto-derive tile_position (0,0)/(64,0)) -> two psum banks
            # of one [P, 2QC] scores tile, running concurrently in the array.
            # PV: col-tiled M=64 pair into ONE bank (rows 0:64 head h0, 64:128
            # h1).  Z: col-tiled M=1 pair into the shared Z bank.
            def attn_scores(pr, sc, kc_, nkc):
                r = kc_ - 4 * sc
                w0 = KC * r if (causal and r > 0) else 0
                ps = ps_s_pool.tile([P, 2 * QC], F32, tag="ps_s", name="ps_s")
                nc.tensor.matmul(
                    ps[:, w0:QC],
                    lhsT=kt[pr][0:A, kc_ * KC:(kc_ + 1) * KC],
                    rhs=qt[pr][0:A, sc * QC + w0:(sc + 1) * QC],
                    start=True, stop=True,
                )
                nc.tensor.matmul(
                    ps[:, QC + w0:2 * QC],
                    lhsT=kt[pr][A:2 * A, kc_ * KC:(kc_ + 1) * KC],
                    rhs=qt[pr][A:2 * A, sc * QC + w0:(sc + 1) * QC],
                    start=True, stop=True,
                )
                ex = ex_pool.tile([P, 2 * QC], DT16, tag="ex", name="ex")
                if w0 == 0:
                    nc.scalar.activation(out=ex[:], in_=ps[:],
                                         func=EXP, scale=SCALE)
                else:
                    nc.scalar.activation(out=ex[:, w0:QC], in_=ps[:, w0:QC],
                                         func=EXP, scale=SCALE)
                    nc.scalar.activation(out=ex[:, QC + w0:2 * QC],
                                         in_=ps[:, QC + w0:2 * QC],
                                         func=EXP, scale=SCALE)
                if causal:
                    if r >= 0:  # mask the 128-wide boundary band only
                        nc.vector.tensor_mul(
                            ex[:, w0:w0 + KC], ex[:, w0:w0 + KC], m01_sb[:])
                        nc.vector.tensor_mul(
                            ex[:, QC + w0:QC + w0 + KC],
                            ex[:, QC + w0:QC + w0 + KC], m01_sb[:])
                else:
                    em = ex_pool.tile([P, QC], DT16, tag="em", name="em")
                    nc.sync.dma_start(
                        out=em[:],
                        in_=emT[:, :][kc_ * KC:(kc_ + 1) * KC,
                                      sc * QC:(sc + 1) * QC],
                    )
                    nc.vector.tensor_mul(ex[:, 0:QC], ex[:, 0:QC], em[:])
                    nc.vector.tensor_mul(ex[:, QC:2 * QC], ex[:, QC:2 * QC], em[:])
                return ex, w0

            def attn_pvz(pr, sc, kc_, nkc, pvA, pvB, ex, w0):
                # PV with ones-column (psum row A holds Z).  Non-tiled M=65
                # matmuls: their LDWEIGHTS hoist through the PE reorder
                # window (tiled matmuls' can't -- weight-slot conflicts cost
                # ~100ns/pass exposed), so serial M=65 beats a col-tiled
                # M=64 pair plus a separate Z pass.
                st, sp = (kc_ == 0), (kc_ == nkc - 1)
                h0, h1 = 2 * pr, 2 * pr + 1
                nc.tensor.matmul(
                    pvA[0:A + 1, w0:QC],
                    lhsT=v_sb[:, kc_, h0 * (A + 1):(h0 + 1) * (A + 1)],
                    rhs=ex[:, w0:QC],
                    start=st, stop=sp,
                )
                nc.tensor.matmul(
                    pvB[0:A + 1, w0:QC],
                    lhsT=v_sb[:, kc_, h1 * (A + 1):(h1 + 1) * (A + 1)],
                    rhs=ex[:, QC + w0:2 * QC],
                    start=st, stop=sp,
                )

            tail_ps = []    # ic0-half psums started during the tail normalize

            def tail_pre(pvA):
                # Start the u_sb[0]-half of ALL eight tail out-projections
                # while the last normalize's Z chain runs (u_sb[0]'s chunk-3
                # half was normalized one stream earlier).  Banks: proj 2 +
                # two scores tiles (2 banks each, via independent column
                # halves) + the tail stream's own pvA; its pvB serves as the
                # normalize's broadcast bank first, then joins as the eighth
                # after the Ln pass frees it (handled by the caller).
                sc = NQC - 1
                sh = {}
                for jc in range(7):
                    off = 0
                    if jc in (0, 1):
                        ps = ps_proj.tile([P, QC], F32, tag="ps_p", name="ps_e")
                    elif jc in (2, 3):
                        ps = ps_s_pool.tile([P, 2 * QC], F32, tag="ps_s",
                                            name="ps_e")
                        sh[jc + 2] = ps
                    elif jc in (4, 5):
                        ps = sh[jc]
                        off = QC
                    else:  # jc == 6
                        ps = pvA
                    nc.tensor.matmul(
                        ps[:, off:off + QC],
                        lhsT=wo_sb[:, 0, jc * P:(jc + 1) * P],
                        rhs=u_sb[0][:, sc * QC:(sc + 1) * QC],
                        start=True, stop=False,
                    )
                    tail_ps.append((ps, off))

            def normalize(pr, sc, pvA, pvB, tail=False):
                if tail:
                    # Critical-path variant: no DRAM bounce.  Evict u halves
                    # on ACT, broadcast Z down 64 partitions with a ones-lhsT
                    # matmul (into the freed pvB bank), 1/Z = exp(-ln Z) on
                    # ACT, multiply.
                    nc.scalar.copy(
                        out=u_sb[pr][0:A, sc * QC:(sc + 1) * QC], in_=pvA[0:A, :])
                    bt = norm_pool.tile([A, QC], DT16, tag="bt", name="bt")
                    nc.scalar.copy(out=bt[:], in_=pvB[0:A, :])
                    nc.scalar.dma_start(
                        out=u_sb[pr][A:2 * A, sc * QC:(sc + 1) * QC], in_=bt[:])
                    zr = norm_pool.tile([P, 2 * QC], DT16, tag="zr", name="zr")
                    nc.vector.tensor_copy(out=zr[A:A + 1, 0:QC], in_=pvA[A:A + 1, :])
                    nc.vector.tensor_copy(out=zr[A:A + 1, QC:2 * QC],
                                          in_=pvB[A:A + 1, :])
                    rbp = pvB[:, 0:QC]
                    nc.tensor.matmul(rbp[0:A, :], lhsT=ones_t[A:A + 1, 0:A],
                                     rhs=zr[A:A + 1, 0:QC], start=True, stop=True)
                    nc.tensor.matmul(rbp[A:2 * A, :], lhsT=ones_t[A:A + 1, 0:A],
                                     rhs=zr[A:A + 1, QC:2 * QC], start=True, stop=True)
                    lnz = norm_pool.tile([P, QC], F32, tag="lnz", name="lnz")
                    nc.scalar.activation(out=lnz[:], in_=rbp[:], func=LN)
                    rb = norm_pool.tile([P, QC], F32, tag="rb", name="rb")
                    nc.scalar.activation(out=rb[:], in_=lnz[:],
                                         func=EXP, scale=-1.0)
                    tail_pre(pvA)
                    # jc7 reuses the broadcast bank once Ln has read it
                    nc.tensor.matmul(
                        pvB[:, 0:QC],
                        lhsT=wo_sb[:, 0, 7 * P:8 * P],
                        rhs=u_sb[0][:, sc * QC:(sc + 1) * QC],
                        start=True, stop=False,
                    )
                    tail_ps.append((pvB, 0))
                    nc.vector.tensor_mul(
                        u_sb[pr][:, sc * QC:(sc + 1) * QC],
                        u_sb[pr][:, sc * QC:(sc + 1) * QC], rb[:])
                    return
                # Evict U unnormalized (frees PV psum fast); 1/Z via DRAM
                # bounce reshaped [128, 8] (lane-parallel reciprocal), then
                # one in-place multiply.  ACT stays exp-only.
                zr = norm_pool.tile([P, 2 * QC], F32, tag="zf", name="zf")
                nc.vector.tensor_copy(out=zr[A:A + 1, 0:QC], in_=pvA[A:A + 1, :])
                nc.vector.tensor_copy(out=zr[A:A + 1, QC:2 * QC], in_=pvB[A:A + 1, :])
                nc.vector.tensor_copy(
                    out=u_sb[pr][0:A, sc * QC:(sc + 1) * QC], in_=pvA[0:A, :])
                bt = norm_pool.tile([A, QC], DT16, tag="bt", name="bt")
                nc.vector.tensor_copy(out=bt[:], in_=pvB[0:A, :])
                nc.gpsimd.dma_start(
                    out=u_sb[pr][A:2 * A, sc * QC:(sc + 1) * QC], in_=bt[:])
                zd = zd_pool.tile([1, 2 * QC], F32, tag="zd", name="zd")
                nc.sync.dma_start(out=zd[:], in_=zr[A:A + 1, :])
                zre = bass.AP(tensor=zd.tensor, offset=zd.offset,
                              ap=[[8, P], [1, 8]])
                zi = norm_pool.tile([P, 8], F32, tag="zi", name="zi")
                nc.sync.dma_start(out=zi[:], in_=zre)
                nc.vector.reciprocal(out=zi[:], in_=zi[:])
                zd2 = zd_pool.tile([1, 2 * QC], F32, tag="zd2", name="zd2")
                zre2 = bass.AP(tensor=zd2.tensor, offset=zd2.offset,
                               ap=[[8, P], [1, 8]])
                nc.sync.dma_start(out=zre2, in_=zi[:])
                rb = norm_pool.tile([P, QC], F32, tag="rb", name="rb")
                nc.sync.dma_start(out=rb[0:A, :], in_=_bcast_part(zd2[0:1, 0:QC], A))
                nc.sync.dma_start(out=rb[A:2 * A, :],
                                  in_=_bcast_part(zd2[0:1, QC:2 * QC], A))
                nc.vector.tensor_mul(
                    u_sb[pr][:, sc * QC:(sc + 1) * QC],
                    u_sb[pr][:, sc * QC:(sc + 1) * QC], rb[:])

            # ---- schedule ---------------------------------------------------
            # Single lane, pr-major ascending chunks.  The exp-heavy chunk-3
            # windows get the deferred out-projections and the just-in-time
            # k/v projections as PE fill.
            cur_streams = [(0, 0), (0, 1), (1, 0), (1, 1), (2, 0), (2, 1),
                           (3, 0), (3, 1)]
            nkc_of = (lambda sc: 4 * sc + 4) if causal else (lambda sc: NKC)

            cur_blocks = []
            for sc, pr in cur_streams:
                for kc_ in range(nkc_of(sc)):
                    cur_blocks.append((sc, pr, kc_))
            n_pos = len(cur_blocks)   # 80 causal / 128 non-causal
            early_at = {}

            # fill units keyed by target position (emitted right after that
            # position's block).  Deadlines honored; oproj(sc) deferred two
            # chunks to plug the later ACT-heavy windows.
            fill_at = {}

            def put(pos, fn):
                fill_at.setdefault(pos, []).append(fn)

            if causal:
                put(0, lambda: v_proj(2))
                put(1, lambda: v_proj(3))
                put(2, lambda: q_proj(1, 0))
                put(3, lambda: k_proj(1, 0))
                put(4, lambda: q_proj(0, 1))
                put(5, lambda: k_proj(0, 1))
                put(6, lambda: v_proj(4))
                put(8, lambda: v_proj(5))
                put(9, lambda: v_proj(6))
                put(10, lambda: v_proj(7))
                put(12, lambda: q_proj(1, 1))
                put(13, lambda: k_proj(1, 1))
                put(16, lambda: q_proj(0, 2))
                put(17, lambda: k_proj(0, 2))
                put(19, lambda: out_proj(0, 0))
                put(21, lambda: out_proj(1, 0))
                put(24, lambda: v_proj(8))
                put(25, lambda: v_proj(9))
                put(26, lambda: v_proj(10))
                put(27, lambda: v_proj(11))
                put(28, lambda: out_proj(2, 0))
                put(29, lambda: out_proj(3, 0))
                put(30, lambda: q_proj(1, 2))
                put(31, lambda: k_proj(1, 2))
                put(32, lambda: out_proj(4, 0))
                put(33, lambda: out_proj(5, 0))
                put(34, lambda: out_proj(6, 0))
                put(35, lambda: out_proj(7, 0))
                put(36, lambda: v_proj(12))
                put(38, lambda: v_proj(13))
                put(40, lambda: v_proj(14))
                put(42, lambda: v_proj(15))
                put(44, lambda: q_proj(0, 3))
                put(45, lambda: out_proj(0, 1))
                put(46, lambda: out_proj(1, 1))
                put(47, lambda: out_proj(2, 1))
                put(48, lambda: k_proj(0, 3))
                put(49, lambda: out_proj(3, 1))
                put(50, lambda: out_proj(4, 1))
                put(52, lambda: out_proj(5, 1))
                put(54, lambda: out_proj(6, 1))
                put(56, lambda: out_proj(7, 1))
                put(57, lambda: q_proj(1, 3))
                put(58, lambda: out_proj(0, 2))
                put(59, lambda: out_proj(1, 2))
                put(60, lambda: out_proj(2, 2))
                put(61, lambda: out_proj(3, 2))
                put(62, lambda: out_proj(4, 2))
                put(63, lambda: out_proj(5, 2))
                put(65, lambda: out_proj(6, 2))
                put(66, lambda: k_proj(1, 3))
                put(67, lambda: out_proj(7, 2))
            else:
                # non-causal correctness path: everything is projected in the
                # prologue (below); only the out-projections ride as fill.
                for jc in range(8):
                    put(34 + jc, lambda jc=jc: out_proj(jc, 0))
                for jc in range(8):
                    put(66 + jc, lambda jc=jc: out_proj(jc, 1))
                for jc in range(8):
                    put(98 + jc, lambda jc=jc: out_proj(jc, 2))

            # Prologue: only what the first blocks need.
            q_proj(0, 0)
            k_proj(0, 0, alt=True)
            v_proj(0)
            v_proj(1, alt=True)
            if not causal:
                for nsc in range(NQC):
                    for mc in range(2):
                        if (mc, nsc) != (0, 0):
                            q_proj(mc, nsc, alt=(nsc % 2 == 0))
                            k_proj(mc, nsc, alt=(nsc % 2 == 1))
                for cc in range(2, NKC):
                    v_proj(cc, alt=(cc % 2 == 0))

            pv_of = {}       # stream -> (pvA, pvB) psum tiles
            pending = []     # deferred PV/normalize closures (FIFO)

            def emit_block(stream, kc_, lane):
                sc, pr = stream
                nkc = nkc_of(sc)
                if kc_ == 0:
                    pv_of[stream] = (
                        ps_pv_pool.tile([P, QC], F32, tag=f"pv{lane}A",
                                        name=f"pv{lane}A"),
                        ps_pv_pool.tile([P, QC], F32, tag=f"pv{lane}B",
                                        name=f"pv{lane}B"),
                    )
                pvA, pvB = pv_of[stream]
                ex, w0 = attn_scores(pr, sc, kc_, nkc)
                pending.append(
                    lambda: attn_pvz(pr, sc, kc_, nkc, pvA, pvB, ex, w0))
                if kc_ == nkc - 1:
                    is_tail = (stream == cur_streams[-1])
                    pending.append(
                        lambda: normalize(pr, sc, pvA, pvB, tail=is_tail))

            for pos, (sc, pr, kc_) in enumerate(cur_blocks):
                emit_block((sc, pr), kc_, "C")
                while len(pending) > 2:
                    pending.pop(0)()
                for fn in fill_at.get(pos, ()):
                    fn()
            while pending:
                pending.pop(0)()

            # Tail: finish the eight out-projections with the u_sb[1] half.
            for jc in range(8):
                ps, off = tail_ps[jc]
                nc.tensor.matmul(
                    ps[:, off:off + QC],
                    lhsT=wo_sb[:, 1, jc * P:(jc + 1) * P],
                    rhs=u_sb[1][:, (NQC - 1) * QC:NQC * QC],
                    start=False, stop=True,
                )
                o_evac(jc, NQC - 1, ps, off, act_ok=True)

    return nc


def _split_waits(nc: bass.Bass) -> int:
    """The walrus build here allows one sync wait per engine instruction;
    Tile emits several.  Hoist extras into standalone single-wait
    EventSemaphore instructions on the same engine queue (in-order, so
    semantics are preserved).  DMACopy waits lower into queue descriptors and
    are left alone."""
    n = 0
    for func in nc.m.functions:
        for block in func.blocks:
            out = []
            for ins in block.instructions:
                si = ins.sync_info
                if si is not None and len(si.on_wait) > 1:
                    waits = list(si.on_wait)
                    for w in waits[:-1]:
                        es = mybir.InstEventSemaphore(
                            name=f"waitsplit_{n}", ins=[], outs=[])
                        n += 1
                        es.engine = ins.engine
                        es.sync_info = type(si)(on_wait=[w], on_update=[])
                        out.append(es)
                    si.on_wait = [waits[-1]]
                    ins.sync_info = si
                out.append(ins)
            block.instructions = out
    return n


def _get_prog(causal: bool) -> bass.Bass:
    if causal not in _prog_cache:
        nc = _build(causal)
        _split_waits(nc)
        _prog_cache[causal] = nc
    return _prog_cache[causal]


def _is_causal(mask: np.ndarray) -> bool:
    if mask.shape != (S, S):
        return False
    tri = np.tril(np.ones((S, S), dtype=bool))
    low = mask[tri]
    up = mask[~tri]
    return bool((low == 0.0).all() and (up <= -1e8).all())


def _m01_patterns() -> np.ndarray:
    # Boundary-band mask: band column j vs partition p -> keep iff j >= p.
    j = np.arange(KC)[None, :]
    p = np.arange(P)[:, None]
    return (j >= p).astype(BF16)


def _prep_in_maps(query, context, Wq, Wkv, Wout, mask, causal):
    query = np.asarray(query, dtype=np.float32)
    context = np.asarray(context, dtype=np.float32)
    Wq = np.asarray(Wq, dtype=np.float32)
    Wkv = np.asarray(Wkv, dtype=np.float32)
    Wout = np.asarray(Wout, dtype=np.float32)

    def sw_act(x):   # [D, S] -> [P, NQC, DC, QC] (sc-major SBUF-layout swizzle)
        return np.ascontiguousarray(
            x.reshape(DC, P, NQC, QC).transpose(1, 2, 0, 3)).astype(BF16)

    def sw_w(w):     # [D, M] -> [P, DC, M]
        return np.ascontiguousarray(
            w.reshape(DC, P, M).transpose(1, 0, 2)).astype(BF16)

    def sw_wo(w):    # [M, D] -> [P, 2, D]
        return np.ascontiguousarray(
            w.reshape(2, P, D).transpose(1, 0, 2)).astype(BF16)

    qT = [sw_act(query[b].T) for b in range(B)]
    cT = [sw_act(context[b].T) for b in range(B)]
    if causal:
        extra = ("m01", _m01_patterns())
    else:
        extra = ("emT", np.exp((SCALE * np.asarray(mask, np.float32).T)).astype(BF16))

    in_maps = []
    for c in range(8):
        b, g = divmod(c, 4)
        m0 = g * M
        in_maps.append({
            "qT": qT[b],
            "cT": cT[b],
            "wqT": sw_w(Wq[m0:m0 + M, :].T),
            "wkT": sw_w(Wkv[m0:m0 + M, :].T),
            "wvT": sw_w(Wkv[D + m0:D + m0 + M, :].T),
            "woT": sw_wo(Wout[:, m0:m0 + M].T),
            extra[0]: extra[1],
        })
    return in_maps


def _run(query, context, Wq, Wkv, Wout, mask, trace=False):
    causal = _is_causal(np.asarray(mask, np.float32))
    in_maps = _prep_in_maps(query, context, Wq, Wkv, Wout, mask, causal)
    nc = _get_prog(causal)
    res = run_bass_kernel_spmd(nc, in_maps, list(range(8)), trace=trace)
    out = np.zeros((B, S, D), dtype=np.float32)
    for c in range(8):
        out[c // 4] += res.results[c]["outT"].T.astype(np.float32)
    return out, res


def kernel(query, context, Wq, Wkv, Wout, mask):
    out, _ = _run(query, context, Wq, Wkv, Wout, mask, trace=False)
    return out


# revision 38
# speedup vs baseline: 1.1919x; 1.0099x over previous
"""Multi-head attention kernel for Trainium2, sharded over 8 NeuronCores.

Sharding: core c handles batch c//4 and heads 4*(c%4) .. 4*(c%4)+4
(data parallel on batch, tensor parallel on heads).  Each core computes a
partial output projection (its heads' slice of Wout); the host sums the 4
partials per batch at gather time.

Self-contained: hardcodes B=2, S=2048, D=1024, H=16.

Schedule: causal attention concentrates exp work (ACT engine) in the last
q-chunk, which made the endgame ACT-bound while the PE idled.  The kernel
therefore runs stream (sc=3, pr=1) EARLY, interleaved one-block-per-~2.5
into chunks 0-2 (its kt/v/qt inputs arrive early enough), so the exp hump
flattens across the whole kernel.  PSUM feasibility comes from packing each
stream's two heads' PV into ONE psum bank via col-tiled M=64 matmul pairs
(tile_position (0,0)/(0,64)) with softmax denominators accumulated as M=1
col-tiled matmuls into a shared Z bank (rows 64/96 = current lane's two
heads, rows 0/32 = early lane's).  Banks: scores 2x[128,2QC]=4 + PV 2 +
Z 1 + proj 1 = 8 exactly.
"""

import numpy as np
import ml_dtypes
from contextlib import ExitStack

import concourse.bass as bass
import concourse.tile as tile
from concourse import mybir
from concourse import bass_utils as _BU
from concourse.bass_utils import run_bass_kernel_spmd

BF16 = ml_dtypes.bfloat16

B, S, D, H = 2, 2048, 1024, 16
A = 64                  # head dim
NH = 4                  # heads per core
M = NH * A              # 256: local projection width
SCALE = 1.0 / 32.0      # 1/sqrt(D)
P = 128
QC = 512                # q chunk (matmul free dim)
NQC = S // QC           # 4
KC = 128                # k chunk (contraction tile for PV)
NKC = S // KC           # 16
DC = D // P             # 8 d-chunks

F32 = mybir.dt.float32
DT16 = mybir.dt.bfloat16
EXP = mybir.ActivationFunctionType.Exp
LN = mybir.ActivationFunctionType.Ln

_prog_cache = {}


def _bcast_part(ap, n):
    """Broadcast a [1, ...] AP across n partitions (step-0 partition dim)."""
    return bass.AP(tensor=ap.tensor, offset=ap.offset, ap=[[0, n]] + list(ap.ap[1:]))


def _build(causal: bool) -> bass.Bass:
    nc = bass.Bass()

    # all inputs pre-swizzled on host into SBUF layout (contiguous per
    # partition -> large DMA descriptors -> near-peak HBM bandwidth).
    qT = nc.dram_tensor("qT", [P, NQC, DC, QC], DT16, kind="ExternalInput")
    cT = nc.dram_tensor("cT", [P, NQC, DC, QC], DT16, kind="ExternalInput")
    wqT = nc.dram_tensor("wqT", [P, DC, M], DT16, kind="ExternalInput")
    wkT = nc.dram_tensor("wkT", [P, DC, M], DT16, kind="ExternalInput")
    wvT = nc.dram_tensor("wvT", [P, DC, M], DT16, kind="ExternalInput")
    woT = nc.dram_tensor("woT", [P, 2, D], DT16, kind="ExternalInput")
    if causal:
        m01 = nc.dram_tensor("m01", [P, KC], DT16, kind="ExternalInput")
    else:
        emT = nc.dram_tensor("emT", [S, S], DT16, kind="ExternalInput")
    outT = nc.dram_tensor("outT", [D, S], DT16, kind="ExternalOutput")

    with tile.TileContext(nc) as tc, ExitStack() as ctx:
        const = ctx.enter_context(tc.tile_pool(name="const", bufs=1))

        # Persistent SBUF tensors
        qt_in = const.tile([P, DC, S], DT16, tag="qt_in")    # query^T  (d on partitions)
        ct_in = const.tile([P, DC, S], DT16, tag="ct_in")    # context^T
        wq_sb = const.tile([P, DC, M], DT16, tag="wq_sb")
        wk_sb = const.tile([P, DC, M], DT16, tag="wk_sb")
        wv_sb = const.tile([P, DC, M], DT16, tag="wv_sb")
        wo_sb = const.tile([P, 2, D], DT16, tag="wo_sb")
        qt = [const.tile([P, S], DT16, tag=f"qt{i}", name=f"qt{i}") for i in range(2)]   # Q^T m-chunks
        kt = [const.tile([P, S], DT16, tag=f"kt{i}", name=f"kt{i}") for i in range(2)]   # K^T m-chunks
        v_sb = const.tile([P, NKC, NH * (A + 1)], DT16, tag="v_sb")   # [V_h | 1] blocks
        u_sb = [const.tile([P, S], DT16, tag=f"u{i}", name=f"u{i}") for i in range(2)]  # normalized attn@V
        if causal:
            m01_sb = const.tile([P, KC], DT16, tag="m01_sb")
        ones_t = const.tile([P, A], DT16, tag="ones_t")   # lhsT for Z broadcast

        # Input DMAs, single (sync) queue in need order.  The ~6 us framework
        # preamble gates the first descriptor; after that the pieces stream at
        # full aggregate bandwidth, so granularity is 2-dc (256 KB) for the
        # sc0 slabs that gate the first projections.  q3 is pulled forward:
        # the early lane (3,1) starts a few blocks in.
        def q_slab(eng, sc_, lo, hi):
            return eng.dma_start(out=qt_in[:, lo:hi, sc_ * QC:(sc_ + 1) * QC],
                                 in_=qT[:, sc_, lo:hi, :])

        def c_slab(eng, sc_, lo, hi):
            return eng.dma_start(out=ct_in[:, lo:hi, sc_ * QC:(sc_ + 1) * QC],
                                 in_=cT[:, sc_, lo:hi, :])

        if causal:
            nc.sync.dma_start(out=m01_sb[:], in_=m01[:, :])
        nc.sync.dma_start(out=wq_sb[:, 0:4, :], in_=wqT[:, 0:4, :])
        q_slab(nc.sync, 0, 0, 2)
        q_slab(nc.sync, 0, 2, 4)
        nc.sync.dma_start(out=wq_sb[:, 4:8, :], in_=wqT[:, 4:8, :])
        nc.sync.dma_start(out=wk_sb[:], in_=wkT[:, :, :])
        q_slab(nc.sync, 0, 4, 6)
        c_slab(nc.sync, 0, 0, 2)
        q_slab(nc.sync, 0, 6, 8)
        c_slab(nc.sync, 0, 2, 4)
        c_slab(nc.sync, 0, 4, 6)
        c_slab(nc.sync, 0, 6, 8)
        nc.sync.dma_start(out=wv_sb[:], in_=wvT[:, :, :])
        q_slab(nc.sync, 3, 0, 8)
        c_slab(nc.sync, 1, 0, 4)
        c_slab(nc.sync, 1, 4, 8)
        q_slab(nc.sync, 1, 0, 8)
        nc.sync.dma_start(out=wo_sb[:], in_=woT[:, :, :])
        c_slab(nc.sync, 2, 0, 8)
        q_slab(nc.sync, 2, 0, 8)
        c_slab(nc.sync, 3, 0, 8)

        nc.vector.memset(v_sb[:], 1.0)  # ones columns for the Z (denominator) trick
        nc.vector.memset(ones_t[:], 1.0)

        # PSUM: scores 2x[128,2QC] = 4 banks + PV pair 2 + proj 2 = 8.
        with tc.tile_pool(name="ps_proj", bufs=2, space="PSUM") as ps_proj, \
             tc.tile_pool(name="ps_s", bufs=2, space="PSUM") as ps_s_pool, \
             tc.tile_pool(name="ps_pv", bufs=1, space="PSUM") as ps_pv_pool, \
             tc.tile_pool(name="expool", bufs=12) as ex_pool, \
             tc.tile_pool(name="zdram", bufs=4, space="DRAM") as zd_pool, \
             tc.tile_pool(name="o_stage", bufs=8) as o_stage, \
             tc.tile_pool(name="norm", bufs=4) as norm_pool:

            def _proj_ps(alt=False):
                # alt: prologue units borrow an idle scores-pool bank so the
                # first q/k/v projections overlap instead of serializing on
                # the two proj banks
                if alt:
                    return ps_s_pool.tile([P, 2 * QC], F32, tag="ps_s",
                                          name="ps_alt")[:, 0:QC]
                return ps_proj.tile([P, QC], F32, tag="ps_p", name="ps_p")

            def q_proj(mc, sc, alt=False):
                ps = _proj_ps(alt)
                for dc_ in range(DC):
                    nc.tensor.matmul(
                        ps[:, 0:QC],
                        lhsT=wq_sb[:, dc_, mc * P:(mc + 1) * P],
                        rhs=qt_in[:, dc_, sc * QC:(sc + 1) * QC],
                        start=(dc_ == 0), stop=(dc_ == DC - 1),
                    )
                nc.vector.tensor_copy(out=qt[mc][:, sc * QC:(sc + 1) * QC], in_=ps[:, 0:QC])

            def k_proj(mc, sc, alt=False):
                ps = _proj_ps(alt)
                for dc_ in range(DC):
                    nc.tensor.matmul(
                        ps[:, 0:QC],
                        lhsT=wk_sb[:, dc_, mc * P:(mc + 1) * P],
                        rhs=ct_in[:, dc_, sc * QC:(sc + 1) * QC],
                        start=(dc_ == 0), stop=(dc_ == DC - 1),
                    )
                nc.vector.tensor_copy(out=kt[mc][:, sc * QC:(sc + 1) * QC], in_=ps[:, 0:QC])

            def v_proj(cc, alt=False):
                ps = _proj_ps(alt)
                for dc_ in range(DC):
                    nc.tensor.matmul(
                        ps[:, 0:M],
                        lhsT=ct_in[:, dc_, cc * P:(cc + 1) * P],
                        rhs=wv_sb[:, dc_, :],
                        start=(dc_ == 0), stop=(dc_ == DC - 1),
                    )
                for h in range(NH):
                    nc.vector.tensor_copy(
                        out=v_sb[:, cc, h * (A + 1):h * (A + 1) + A],
                        in_=ps[:, h * A:(h + 1) * A],
                    )

            def o_evac(jc, sc, ps, off=0, act_ok=False):
                ps = ps[:, off:off + QC]
                o_sb = o_stage.tile([P, QC], DT16, tag="o_sb")
                if act_ok:
                    # late windows: ACT has slack, DVE is the busier engine
                    if jc % 2 == 0:
                        nc.scalar.copy(out=o_sb[:], in_=ps[:, 0:QC])
                    else:
                        nc.vector.tensor_copy(out=o_sb[:], in_=ps[:, 0:QC])
                    deng = nc.sync if jc % 2 == 0 else nc.scalar
                else:
                    nc.vector.tensor_copy(out=o_sb[:], in_=ps[:, 0:QC])
                    deng = nc.sync if jc % 2 == 0 else nc.gpsimd
                deng.dma_start(
                    out=outT[:, :][jc * P:(jc + 1) * P, sc * QC:(sc + 1) * QC],
                    in_=o_sb[:])

            def out_proj(jc, sc):
                ps = _proj_ps()
                for ic in range(2):
                    nc.tensor.matmul(
                        ps[:, 0:QC],
                        lhsT=wo_sb[:, ic, jc * P:(jc + 1) * P],
                        rhs=u_sb[ic][:, sc * QC:(sc + 1) * QC],
                        start=(ic == 0), stop=(ic == 1),
                    )
                o_evac(jc, sc, ps)

            # ---- attention blocks -------------------------------------------
            # scores: row-tiled pair (contraction A=64; lhsT base partitions 0
            # and 64 auto-derive tile_position (0,0)/(64,0)) -> two psum banks
            # of one [P, 2QC] scores tile, running concurrently in the array.
            # PV: col-tiled M=64 pair into ONE bank (rows 0:64 head h0, 64:128
            # h1).  Z: col-tiled M=1 pair into the shared Z bank.
            def attn_scores(pr, sc, kc_, nkc):
                r = kc_ - 4 * sc
                w0 = KC * r if (causal and r > 0) else 0
                ps = ps_s_pool.tile([P, 2 * QC], F32, tag="ps_s", name="ps_s")
                nc.tensor.matmul(
                    ps[:, w0:QC],
                    lhsT=kt[pr][0:A, kc_ * KC:(kc_ + 1) * KC],
                    rhs=qt[pr][0:A, sc * QC + w0:(sc + 1) * QC],
                    start=True, stop=True,
                )
                nc.tensor.matmul(
                    ps[:, QC + w0:2 * QC],
                    lhsT=kt[pr][A:2 * A, kc_ * KC:(kc_ + 1) * KC],
                    rhs=qt[pr][A:2 * A, sc * QC + w0:(sc + 1) * QC],
                    start=True, stop=True,
                )
                ex = ex_pool.tile([P, 2 * QC], DT16, tag="ex", name="ex")
                if w0 == 0:
                    nc.scalar.activation(out=ex[:], in_=ps[:],
                                         func=EXP, scale=SCALE)
                else:
                    nc.scalar.activation(out=ex[:, w0:QC], in_=ps[:, w0:QC],
                                         func=EXP, scale=SCALE)
                    nc.scalar.activation(out=ex[:, QC + w0:2 * QC],
                                         in_=ps[:, QC + w0:2 * QC],
                                         func=EXP, scale=SCALE)
                if causal:
                    if r >= 0:  # mask the 128-wide boundary band only
                        nc.vector.tensor_mul(
                            ex[:, w0:w0 + KC], ex[:, w0:w0 + KC], m01_sb[:])
                        nc.vector.tensor_mul(
                            ex[:, QC + w0:QC + w0 + KC],
                            ex[:, QC + w0:QC + w0 + KC], m01_sb[:])
                else:
                    em = ex_pool.tile([P, QC], DT16, tag="em", name="em")
                    nc.sync.dma_start(
                        out=em[:],
                        in_=emT[:, :][kc_ * KC:(kc_ + 1) * KC,
                                      sc * QC:(sc + 1) * QC],
                    )
                    nc.vector.tensor_mul(ex[:, 0:QC], ex[:, 0:QC], em[:])
                    nc.vector.tensor_mul(ex[:, QC:2 * QC], ex[:, QC:2 * QC], em[:])
                return ex, w0

            def attn_pvz(pr, sc, kc_, st, sp, pvA, pvB, ex, w0):
                # PV with ones-column (psum row A holds Z).  Non-tiled M=65
                # matmuls: their LDWEIGHTS hoist through the PE reorder
                # window (tiled matmuls' can't -- weight-slot conflicts cost
                # ~100ns/pass exposed), so serial M=65 beats a col-tiled
                # M=64 pair plus a separate Z pass.
                h0, h1 = 2 * pr, 2 * pr + 1
                nc.tensor.matmul(
                    pvA[0:A + 1, w0:QC],
                    lhsT=v_sb[:, kc_, h0 * (A + 1):(h0 + 1) * (A + 1)],
                    rhs=ex[:, w0:QC],
                    start=st, stop=sp,
                )
                nc.tensor.matmul(
                    pvB[0:A + 1, w0:QC],
                    lhsT=v_sb[:, kc_, h1 * (A + 1):(h1 + 1) * (A + 1)],
                    rhs=ex[:, QC + w0:2 * QC],
                    start=st, stop=sp,
                )

            tail_ps = {}    # jc -> ic0-half psum started before the finish

            def tail_pre_early(jc):
                # proj-bank prestarts, emitted as fill during the tail
                # stream's last (fill-less) blocks to plug the drain bubbles
                sc = NQC - 1
                ps = ps_proj.tile([P, QC], F32, tag="ps_p", name="ps_e")
                nc.tensor.matmul(
                    ps[:, 0:QC],
                    lhsT=wo_sb[:, 0, jc * P:(jc + 1) * P],
                    rhs=u_sb[0][:, sc * QC:(sc + 1) * QC],
                    start=True, stop=False,
                )
                tail_ps[jc] = (ps, 0)

            def tail_pre(pvA):
                # Start the u_sb[0]-half of ALL eight tail out-projections
                # while the last normalize's Z chain runs (u_sb[0]'s chunk-3
                # half was normalized one stream earlier).  Banks: proj 2 +
                # two scores tiles (2 banks each, via independent column
                # halves) + the tail stream's own pvA; its pvB serves as the
                # normalize's broadcast bank first, then joins as the eighth
                # after the Ln pass frees it (handled by the caller).
                sc = NQC - 1
                sh = {}
                for jc in range(2, 7):
                    off = 0
                    if jc in (2, 3):
                        ps = ps_s_pool.tile([P, 2 * QC], F32, tag="ps_s",
                                            name="ps_e")
                        sh[jc + 2] = ps
                    elif jc in (4, 5):
                        ps = sh[jc]
                        off = QC
                    else:  # jc == 6
                        ps = pvA
                    nc.tensor.matmul(
                        ps[:, off:off + QC],
                        lhsT=wo_sb[:, 0, jc * P:(jc + 1) * P],
                        rhs=u_sb[0][:, sc * QC:(sc + 1) * QC],
                        start=True, stop=False,
                    )
                    tail_ps[jc] = (ps, off)

            def normalize(pr, sc, pvA, pvB, tail=False):
                if tail:
                    # Critical-path variant: no DRAM bounce.  Evict u halves
                    # on ACT, broadcast Z down 64 partitions with a ones-lhsT
                    # matmul (into the freed pvB bank), 1/Z = exp(-ln Z) on
                    # ACT, multiply.
                    nc.scalar.copy(
                        out=u_sb[pr][0:A, sc * QC:(sc + 1) * QC], in_=pvA[0:A, :])
                    bt = norm_pool.tile([A, QC], DT16, tag="bt", name="bt")
                    nc.scalar.copy(out=bt[:], in_=pvB[0:A, :])
                    nc.scalar.dma_start(
                        out=u_sb[pr][A:2 * A, sc * QC:(sc + 1) * QC], in_=bt[:])
                    zr = norm_pool.tile([P, 2 * QC], DT16, tag="zr", name="zr")
                    nc.vector.tensor_copy(out=zr[A:A + 1, 0:QC], in_=pvA[A:A + 1, :])
                    nc.vector.tensor_copy(out=zr[A:A + 1, QC:2 * QC],
                                          in_=pvB[A:A + 1, :])
                    rbp = pvB[:, 0:QC]
                    nc.tensor.matmul(rbp[0:A, :], lhsT=ones_t[A:A + 1, 0:A],
                                     rhs=zr[A:A + 1, 0:QC], start=True, stop=True)
                    nc.tensor.matmul(rbp[A:2 * A, :], lhsT=ones_t[A:A + 1, 0:A],
                                     rhs=zr[A:A + 1, QC:2 * QC], start=True, stop=True)
                    lnz = norm_pool.tile([P, QC], F32, tag="lnz", name="lnz")
                    nc.scalar.activation(out=lnz[:], in_=rbp[:], func=LN)
                    rb = norm_pool.tile([P, QC], F32, tag="rb", name="rb")
                    nc.scalar.activation(out=rb[:], in_=lnz[:],
                                         func=EXP, scale=-1.0)
                    tail_pre(pvA)
                    # jc7 reuses the broadcast bank once Ln has read it
                    nc.tensor.matmul(
                        pvB[:, 0:QC],
                        lhsT=wo_sb[:, 0, 7 * P:8 * P],
                        rhs=u_sb[0][:, sc * QC:(sc + 1) * QC],
                        start=True, stop=False,
                    )
                    tail_ps[7] = (pvB, 0)
                    nc.vector.tensor_mul(
                        u_sb[pr][:, sc * QC:(sc + 1) * QC],
                        u_sb[pr][:, sc * QC:(sc + 1) * QC], rb[:])
                    return
                # Evict U unnormalized (frees PV psum fast); 1/Z via DRAM
                # bounce reshaped [128, 8] (lane-parallel reciprocal), then
                # one in-place multiply.  ACT stays exp-only.
                zr = norm_pool.tile([P, 2 * QC], F32, tag="zf", name="zf")
                nc.vector.tensor_copy(out=zr[A:A + 1, 0:QC], in_=pvA[A:A + 1, :])
                nc.vector.tensor_copy(out=zr[A:A + 1, QC:2 * QC], in_=pvB[A:A + 1, :])
                nc.vector.tensor_copy(
                    out=u_sb[pr][0:A, sc * QC:(sc + 1) * QC], in_=pvA[0:A, :])
                bt = norm_pool.tile([A, QC], DT16, tag="bt", name="bt")
                nc.vector.tensor_copy(out=bt[:], in_=pvB[0:A, :])
                nc.gpsimd.dma_start(
                    out=u_sb[pr][A:2 * A, sc * QC:(sc + 1) * QC], in_=bt[:])
                zd = zd_pool.tile([1, 2 * QC], F32, tag="zd", name="zd")
                nc.sync.dma_start(out=zd[:], in_=zr[A:A + 1, :])
                zre = bass.AP(tensor=zd.tensor, offset=zd.offset,
                              ap=[[8, P], [1, 8]])
                zi = norm_pool.tile([P, 8], F32, tag="zi", name="zi")
                nc.sync.dma_start(out=zi[:], in_=zre)
                nc.vector.reciprocal(out=zi[:], in_=zi[:])
                zd2 = zd_pool.tile([1, 2 * QC], F32, tag="zd2", name="zd2")
                zre2 = bass.AP(tensor=zd2.tensor, offset=zd2.offset,
                               ap=[[8, P], [1, 8]])
                nc.sync.dma_start(out=zre2, in_=zi[:])
                rb = norm_pool.tile([P, QC], F32, tag="rb", name="rb")
                nc.sync.dma_start(out=rb[0:A, :], in_=_bcast_part(zd2[0:1, 0:QC], A))
                nc.sync.dma_start(out=rb[A:2 * A, :],
                                  in_=_bcast_part(zd2[0:1, QC:2 * QC], A))
                nc.vector.tensor_mul(
                    u_sb[pr][:, sc * QC:(sc + 1) * QC],
                    u_sb[pr][:, sc * QC:(sc + 1) * QC], rb[:])

            # ---- schedule ---------------------------------------------------
            # Single lane, pr-major ascending chunks.  The exp-heavy chunk-3
            # windows get the deferred out-projections and the just-in-time
            # k/v projections as PE fill.
            cur_streams = [(0, 0), (0, 1), (1, 0), (1, 1), (2, 0), (2, 1),
                           (3, 0), (3, 1)]
            nkc_of = (lambda sc: 4 * sc + 4) if causal else (lambda sc: NKC)

            cur_blocks = []
            for sc, pr in cur_streams:
                kcs = list(range(nkc_of(sc)))
                if causal and (sc, pr) == cur_streams[-1]:
                    # tail stream: narrowest (diagonal) blocks last, so the
                    # final exp->PV chain on the critical path is short and
                    # the drain leaves no PE bubbles.  PV accumulation order
                    # over kc is arbitrary; start/stop flags follow emission.
                    kcs = kcs[:-3] + [kcs[-1], kcs[-2], kcs[-3]]
                for i, kc_ in enumerate(kcs):
                    cur_blocks.append((sc, pr, kc_, i == 0, i == len(kcs) - 1))
            n_pos = len(cur_blocks)   # 80 causal / 128 non-causal
            early_at = {}

            # fill units keyed by target position (emitted right after that
            # position's block).  Deadlines honored; oproj(sc) deferred two
            # chunks to plug the later ACT-heavy windows.
            fill_at = {}

            def put(pos, fn):
                fill_at.setdefault(pos, []).append(fn)

            if causal:
                put(0, lambda: v_proj(2))
                put(1, lambda: v_proj(3))
                put(2, lambda: q_proj(1, 0))
                put(3, lambda: k_proj(1, 0))
                put(4, lambda: q_proj(0, 1))
                put(5, lambda: k_proj(0, 1))
                put(6, lambda: v_proj(4))
                put(8, lambda: v_proj(5))
                put(9, lambda: v_proj(6))
                put(10, lambda: v_proj(7))
                put(12, lambda: q_proj(1, 1))
                put(13, lambda: k_proj(1, 1))
                put(16, lambda: q_proj(0, 2))
                put(17, lambda: k_proj(0, 2))
                put(19, lambda: out_proj(0, 0))
                put(21, lambda: out_proj(1, 0))
                put(24, lambda: v_proj(8))
                put(25, lambda: v_proj(9))
                put(26, lambda: v_proj(10))
                put(27, lambda: v_proj(11))
                put(28, lambda: out_proj(2, 0))
                put(29, lambda: out_proj(3, 0))
                put(30, lambda: q_proj(1, 2))
                put(31, lambda: k_proj(1, 2))
                put(32, lambda: out_proj(4, 0))
                put(33, lambda: out_proj(5, 0))
                put(34, lambda: out_proj(6, 0))
                put(35, lambda: out_proj(7, 0))
                put(36, lambda: v_proj(12))
                put(38, lambda: v_proj(13))
                put(40, lambda: v_proj(14))
                put(42, lambda: v_proj(15))
                put(44, lambda: q_proj(0, 3))
                put(45, lambda: out_proj(0, 1))
                put(46, lambda: out_proj(1, 1))
                put(47, lambda: out_proj(2, 1))
                put(48, lambda: k_proj(0, 3))
                put(49, lambda: out_proj(3, 1))
                put(50, lambda: out_proj(4, 1))
                put(52, lambda: out_proj(5, 1))
                put(54, lambda: out_proj(6, 1))
                put(56, lambda: out_proj(7, 1))
                put(57, lambda: q_proj(1, 3))
                put(58, lambda: out_proj(0, 2))
                put(59, lambda: out_proj(1, 2))
                put(60, lambda: out_proj(2, 2))
                put(61, lambda: out_proj(3, 2))
                put(62, lambda: out_proj(4, 2))
                put(63, lambda: out_proj(5, 2))
                put(65, lambda: out_proj(6, 2))
                put(66, lambda: k_proj(1, 3))
                put(67, lambda: out_proj(7, 2))
            else:
                # non-causal correctness path: everything is projected in the
                # prologue (below); only the out-projections ride as fill.
                for jc in range(8):
                    put(34 + jc, lambda jc=jc: out_proj(jc, 0))
                for jc in range(8):
                    put(66 + jc, lambda jc=jc: out_proj(jc, 1))
                for jc in range(8):
                    put(98 + jc, lambda jc=jc: out_proj(jc, 2))

            put(n_pos - 3, lambda: tail_pre_early(0))
            put(n_pos - 2, lambda: tail_pre_early(1))

            # Prologue: only what the first blocks need.
            q_proj(0, 0)
            k_proj(0, 0, alt=True)
            v_proj(0)
            v_proj(1, alt=True)
            if not causal:
                for nsc in range(NQC):
                    for mc in range(2):
                        if (mc, nsc) != (0, 0):
                            q_proj(mc, nsc, alt=(nsc % 2 == 0))
                            k_proj(mc, nsc, alt=(nsc % 2 == 1))
                for cc in range(2, NKC):
                    v_proj(cc, alt=(cc % 2 == 0))

            pv_of = {}       # stream -> (pvA, pvB) psum tiles
            pending = []     # deferred PV/normalize closures (FIFO)

            def emit_block(stream, kc_, st, sp, lane):
                sc, pr = stream
                nkc = nkc_of(sc)
                if st:
                    pv_of[stream] = (
                        ps_pv_pool.tile([P, QC], F32, tag=f"pv{lane}A",
                                        name=f"pv{lane}A"),
                        ps_pv_pool.tile([P, QC], F32, tag=f"pv{lane}B",
                                        name=f"pv{lane}B"),
                    )
                pvA, pvB = pv_of[stream]
                ex, w0 = attn_scores(pr, sc, kc_, nkc)
                pending.append(
                    lambda: attn_pvz(pr, sc, kc_, st, sp, pvA, pvB, ex, w0))
                if sp:
                    is_tail = (stream == cur_streams[-1])
                    pending.append(
                        lambda: normalize(pr, sc, pvA, pvB, tail=is_tail))

            for pos, (sc, pr, kc_, st, sp) in enumerate(cur_blocks):
                emit_block((sc, pr), kc_, st, sp, "C")
                while len(pending) > 2:
                    pending.pop(0)()
                for fn in fill_at.get(pos, ()):
                    fn()
            while pending:
                pending.pop(0)()

            # Tail: finish the eight out-projections with the u_sb[1] half.
            for jc in range(8):
                ps, off = tail_ps[jc]
                nc.tensor.matmul(
                    ps[:, off:off + QC],
                    lhsT=wo_sb[:, 1, jc * P:(jc + 1) * P],
                    rhs=u_sb[1][:, (NQC - 1) * QC:NQC * QC],
                    start=False, stop=True,
                )
                o_evac(jc, NQC - 1, ps, off, act_ok=True)

    return nc


def _split_waits(nc: bass.Bass) -> int:
    """The walrus build here allows one sync wait per engine instruction;
    Tile emits several.  Hoist extras into standalone single-wait
    EventSemaphore instructions on the same engine queue (in-order, so
    semantics are preserved).  DMACopy waits lower into queue descriptors and
    are left alone."""
    n = 0
    for func in nc.m.functions:
        for block in func.blocks:
            out = []
            for ins in block.instructions:
                si = ins.sync_info
                if si is not None and len(si.on_wait) > 1:
                    waits = list(si.on_wait)
                    for w in waits[:-1]:
                        es = mybir.InstEventSemaphore(
                            name=f"waitsplit_{n}", ins=[], outs=[])
                        n += 1
                        es.engine = ins.engine
                        es.sync_info = type(si)(on_wait=[w], on_update=[])
                        out.append(es)
                    si.on_wait = [waits[-1]]
                    ins.sync_info = si
                out.append(ins)
            block.instructions = out
    return n


def _get_prog(causal: bool) -> bass.Bass:
    if causal not in _prog_cache:
        nc = _build(causal)
        _split_waits(nc)
        _prog_cache[causal] = nc
    return _prog_cache[causal]


def _is_causal(mask: np.ndarray) -> bool:
    if mask.shape != (S, S):
        return False
    tri = np.tril(np.ones((S, S), dtype=bool))
    low = mask[tri]
    up = mask[~tri]
    return bool((low == 0.0).all() and (up <= -1e8).all())


def _m01_patterns() -> np.ndarray:
    # Boundary-band mask: band column j vs partition p -> keep iff j >= p.
    j = np.arange(KC)[None, :]
    p = np.arange(P)[:, None]
    return (j >= p).astype(BF16)


def _prep_in_maps(query, context, Wq, Wkv, Wout, mask, causal):
    query = np.asarray(query, dtype=np.float32)
    context = np.asarray(context, dtype=np.float32)
    Wq = np.asarray(Wq, dtype=np.float32)
    Wkv = np.asarray(Wkv, dtype=np.float32)
    Wout = np.asarray(Wout, dtype=np.float32)

    def sw_act(x):   # [D, S] -> [P, NQC, DC, QC] (sc-major SBUF-layout swizzle)
        return np.ascontiguousarray(
            x.reshape(DC, P, NQC, QC).transpose(1, 2, 0, 3)).astype(BF16)

    def sw_w(w):     # [D, M] -> [P, DC, M]
        return np.ascontiguousarray(
            w.reshape(DC, P, M).transpose(1, 0, 2)).astype(BF16)

    def sw_wo(w):    # [M, D] -> [P, 2, D]
        return np.ascontiguousarray(
            w.reshape(2, P, D).transpose(1, 0, 2)).astype(BF16)

    qT = [sw_act(query[b].T) for b in range(B)]
    cT = [sw_act(context[b].T) for b in range(B)]
    if causal:
        extra = ("m01", _m01_patterns())
    else:
        extra = ("emT", np.exp((SCALE * np.asarray(mask, np.float32).T)).astype(BF16))

    in_maps = []
    for c in range(8):
        b, g = divmod(c, 4)
        m0 = g * M
        in_maps.append({
            "qT": qT[b],
            "cT": cT[b],
            "wqT": sw_w(Wq[m0:m0 + M, :].T),
            "wkT": sw_w(Wkv[m0:m0 + M, :].T),
            "wvT": sw_w(Wkv[D + m0:D + m0 + M, :].T),
            "woT": sw_wo(Wout[:, m0:m0 + M].T),
            extra[0]: extra[1],
        })
    return in_maps


def _run(query, context, Wq, Wkv, Wout, mask, trace=False):
    causal = _is_causal(np.asarray(mask, np.float32))
    in_maps = _prep_in_maps(query, context, Wq, Wkv, Wout, mask, causal)
    nc = _get_prog(causal)
    res = run_bass_kernel_spmd(nc, in_maps, list(range(8)), trace=trace)
    out = np.zeros((B, S, D), dtype=np.float32)
    for c in range(8):
        out[c // 4] += res.results[c]["outT"].T.astype(np.float32)
    return out, res


def kernel(query, context, Wq, Wkv, Wout, mask):
    out, _ = _run(query, context, Wq, Wkv, Wout, mask, trace=False)
    return out
